# revision 6
# baseline (speedup 1.0000x reference)
"""Trainium2 Bass kernel for nn_CrossPixContrastiveL2.

Per sample (one per NeuronCore, N=8 samples / 8 cores):
  dist[p,q] = ||r_p||^2 + ||i_q||^2 - 2 r_p.i_q          (HW x HW, C=128)
  logit = exp(exp(-dist)/10)
  row[p] = sum_q logit*mask / (sum_q logit + eps)
  col[q] = sum_p logit*mask / (sum_p logit + eps)
  loss = masked mean of -log over foreground/nonzero entries

Device strategy per core:
  - bf16 Gram matmuls (K=C=128) into PSUM. A K=2 "broadcast" matmul first
    seeds PSUM with -||i_q||^2/2 (hi/lo bf16 split for f32-level accuracy),
    so PSUM = r.i - ||i||^2/2. The -||r_p||^2 term enters as the per-
    partition f32 bias of the first ACT pass.
  - ACT pass 1: e1 = Exp(2*PSUM - ||r||^2)   -> exp(-dist), bf16
  - ACT pass 2: logit = Exp(e1/10), fused accum_out -> row sums of logit
  - DVE scalar_tensor_tensor: (im_bcast == rm[p]) * logit with fused
    accum_out -> masked row sums (single op: mask+mul+reduce)
  - Column sums via label-onehot matmuls: lhsT = [onehot(rm) | ones]
    (128 x 22) against logit, accumulated over row tiles -> per-label
    column masses T[l,q]; col_lm[q] = T[im[q],q] via elementwise onehot
    select + K=22 matmul.
Host does the tiny final -log / masked mean over 4x1024 values per core.
"""

import os
from contextlib import ExitStack

import numpy as np
import ml_dtypes

import concourse.bacc as bacc
import concourse.tile as tile
import concourse.mybir as mybir
from concourse.bass_utils import run_bass_kernel_spmd

N, C, H, W = 8, 128, 32, 32
HW = H * W
NCORES = 8
NK = HW // 128          # 8 row tiles of 128 pixels
L = 21                  # label values 0..20
LL = L + 1              # onehot columns + ones column
TEMPERATURE = 10.0
EPS = 1e-6

_BF16 = ml_dtypes.bfloat16

_PROGRAM = None


def _build_program():
    f32 = mybir.dt.float32
    bf16 = mybir.dt.bfloat16
    AF = mybir.ActivationFunctionType
    ALU = mybir.AluOpType

    nc = bacc.Bacc("TRN2", target_bir_lowering=False, debug=False,
                   num_devices=NCORES)

    rgb = nc.dram_tensor("rgb", (C, HW), bf16, kind="ExternalInput").ap()
    irr = nc.dram_tensor("irr", (C, HW), bf16, kind="ExternalInput").ap()
    # hi/lo bf16 split of -||i_q||^2/2 (row0=hi, row1=lo)
    nihb = nc.dram_tensor("nihb", (2, HW), bf16, kind="ExternalInput").ap()
    # two rows of ones (lhsT for the K=2 broadcast matmul)
    ones2 = nc.dram_tensor("ones2", (2, 128), bf16, kind="ExternalInput").ap()
    # -||r_p||^2 in transposed layout [p, k] (ACT bias, f32 exact)
    nrT = nc.dram_tensor("nrT", (128, NK), f32, kind="ExternalInput").ap()
    # ir labels broadcast across partitions (bf16, exact for ints 0..20)
    imb = nc.dram_tensor("imb", (128, HW), bf16, kind="ExternalInput").ap()
    # [p, LL*k + l] = (rm[128k+p] == l) for l<21 ; 1.0 at l=21
    oh = nc.dram_tensor("oh", (128, NK * LL), bf16, kind="ExternalInput").ap()
    # rows 0..20 = onehot of im along q; row 21 = ones
    ohim = nc.dram_tensor("ohim", (LL, HW), f32, kind="ExternalInput").ap()
    # rm labels, transposed layout: [p, k] = rm[128k+p]
    rmf = nc.dram_tensor("rmf", (128, NK), f32, kind="ExternalInput").ap()
    # selector (22,2): col0 = 21 ones then 0 ; col1 = zeros then 1
    sel = nc.dram_tensor("sel", (LL, 2), f32, kind="ExternalInput").ap()

    # outputs: rows[:, 0:NK] = masked row sums, rows[:, NK:2NK] = row sums
    rows = nc.dram_tensor("rows", (128, 2 * NK), f32,
                          kind="ExternalOutput").ap()
    # cols[0] = masked col sums, cols[1] = col sums
    cols = nc.dram_tensor("cols", (2, HW), f32, kind="ExternalOutput").ap()

    with tile.TileContext(nc) as tc, ExitStack() as ctx:
        sb = ctx.enter_context(tc.tile_pool(name="sb", bufs=1))
        work = ctx.enter_context(tc.tile_pool(name="work", bufs=3))
        ps = ctx.enter_context(tc.tile_pool(name="ps", bufs=3, space="PSUM"))
        acc = ctx.enter_context(tc.tile_pool(name="acc", bufs=1, space="PSUM"))

        # ---- HAM warm-up: keep the PE busy while input DMAs stream so the
        # real matmuls run at 2.4 GHz instead of the cold 1.2 GHz.
        wz_l = sb.tile([2, 128], bf16)
        nc.gpsimd.memset(wz_l[:], 0.0)
        wz_r = sb.tile([2, 512], bf16)
        nc.gpsimd.memset(wz_r[:], 0.0)
        warm = acc.tile([LL, HW], f32, tag="TT")
        for _ in range(28):
            nc.tensor.matmul(warm[:2, 0:512], wz_l[:, 0:2], wz_r[:],
                             start=True, stop=True)

        # ---- inputs; spread the DMA issues across idle engine queues and
        # order them by when the compute first needs each tensor.
        ir_s = sb.tile([C, HW], bf16)
        nc.sync.dma_start(ir_s[:, 0:512], irr[:, 0:512])
        nc.sync.dma_start(ir_s[:, 512:], irr[:, 512:])
        rgb_s = sb.tile([C, HW], bf16)
        nc.gpsimd.dma_start(rgb_s[:, 0:256], rgb[:, 0:256])
        nc.gpsimd.dma_start(rgb_s[:, 256:], rgb[:, 256:])
        nihb_s = sb.tile([2, HW], bf16)
        nc.sync.dma_start(nihb_s[:], nihb)
        ones2_s = sb.tile([2, 128], bf16)
        nc.sync.dma_start(ones2_s[:], ones2)
        nrT_s = sb.tile([128, NK], f32)
        nc.sync.dma_start(nrT_s[:], nrT)
        imb_s = sb.tile([128, HW], bf16)
        nc.scalar.dma_start(imb_s[:], imb)
        rmf_s = sb.tile([128, NK], f32)
        nc.gpsimd.dma_start(rmf_s[:], rmf)
        oh_s = sb.tile([128, NK * LL], bf16)
        nc.scalar.dma_start(oh_s[:], oh)
        ohim_s = sb.tile([LL, HW], f32)
        nc.scalar.dma_start(ohim_s[:], ohim)
        sel_s = sb.tile([LL, 2], f32)
        nc.scalar.dma_start(sel_s[:], sel)

        rows_s = sb.tile([128, 2 * NK], f32)
        # per-label column masses, accumulated across the NK row tiles
        TT = acc.tile([LL, HW], f32, tag="TT")

        for k in range(NK):
            G = ps.tile([128, HW], f32)
            for qh in range(2):
                q = qh * 512
                # PSUM <- -||i||^2/2 broadcast (K=2 hi/lo), then += r.i
                nc.tensor.matmul(G[:, q:q + 512],
                                 ones2_s[:],
                                 nihb_s[:, q:q + 512],
                                 start=True, stop=False)
                nc.tensor.matmul(G[:, q:q + 512],
                                 rgb_s[:, k * 128:(k + 1) * 128],
                                 ir_s[:, q:q + 512],
                                 start=False, stop=True)
            e1 = work.tile([128, HW], bf16, tag="e1")
            nc.scalar.activation(e1[:], G[:], AF.Exp, scale=2.0,
                                 bias=nrT_s[:, k:k + 1])
            logit = work.tile([128, HW], bf16, tag="logit")
            nc.scalar.activation(logit[:], e1[:], AF.Exp,
                                 scale=1.0 / TEMPERATURE,
                                 accum_out=rows_s[:, NK + k:NK + k + 1])
            lm = work.tile([128, HW], bf16, tag="lm")
            nc.vector.scalar_tensor_tensor(
                lm[:], imb_s[:], rmf_s[:, k:k + 1], logit[:],
                op0=ALU.is_equal, op1=ALU.mult,
                accum_out=rows_s[:, k:k + 1])
            for qh in range(2):
                q = qh * 512
                nc.tensor.matmul(TT[:, q:q + 512],
                                 oh_s[:, LL * k:LL * (k + 1)],
                                 logit[:, q:q + 512],
                                 start=(k == 0), stop=(k == NK - 1))

        # col_lm[q] = TT[im[q], q]; col_logit[q] = TT[21, q]
        TM = sb.tile([LL, HW], f32)
        nc.vector.tensor_tensor(TM[:], TT[:], ohim_s[:], op=ALU.mult)
        cps = acc.tile([2, HW], f32, tag="TT")
        for qh in range(2):
            q = qh * 512
            nc.tensor.matmul(cps[:, q:q + 512], sel_s[:], TM[:, q:q + 512],
                             start=True, stop=True)
        cols_s = sb.tile([2, HW], f32)
        nc.vector.tensor_copy(cols_s[:], cps[:])

        nc.sync.dma_start(rows, rows_s[:])
        nc.sync.dma_start(cols, cols_s[:])

    nc.compile()
    return nc


def _get_program():
    global _PROGRAM
    if _PROGRAM is None:
        _PROGRAM = _build_program()
    return _PROGRAM


def _make_in_map(rgb_map, ir_map, rgb_mask, ir_mask, n):
    f32 = np.float32
    rgb32 = np.ascontiguousarray(rgb_map[n].reshape(C, HW), dtype=f32)
    irr32 = np.ascontiguousarray(ir_map[n].reshape(C, HW), dtype=f32)
    rm = rgb_mask[n].reshape(HW)
    im = ir_mask[n].reshape(HW)

    nr = (rgb32 * rgb32).sum(axis=0, dtype=f32)
    ni = (irr32 * irr32).sum(axis=0, dtype=f32)

    x = (-0.5 * ni).astype(f32)
    hi = x.astype(_BF16)
    lo = (x - hi.astype(f32)).astype(_BF16)
    nihb = np.stack([hi, lo])

    ones2 = np.ones((2, 128), dtype=_BF16)

    rmT = rm.reshape(NK, 128).T  # [p, k]
    nrT = np.ascontiguousarray(-nr.reshape(NK, 128).T, dtype=f32)

    imb = np.broadcast_to(im.astype(_BF16), (128, HW)).copy()

    oh = np.zeros((128, NK, LL), dtype=_BF16)
    oh[:, :, :L] = (rmT[:, :, None] == np.arange(L)[None, None, :])
    oh[:, :, L] = 1
    oh = oh.reshape(128, NK * LL)

    ohim = np.zeros((LL, HW), dtype=f32)
    ohim[:L] = (np.arange(L)[:, None] == im[None, :])
    ohim[L] = 1.0

    rmf = np.ascontiguousarray(rmT, dtype=f32)

    sel = np.zeros((LL, 2), dtype=f32)
    sel[:L, 0] = 1.0
    sel[L, 1] = 1.0

    return {"rgb": rgb32.astype(_BF16), "irr": irr32.astype(_BF16),
            "nihb": nihb, "ones2": ones2, "nrT": nrT, "imb": imb,
            "oh": oh, "ohim": ohim, "rmf": rmf, "sel": sel}


def run_device(rgb_map, ir_map, rgb_mask, ir_mask, trace=False, **trace_kw):
    """Compile+run the SPMD kernel; returns (per-core results, BassKernelResults)."""
    nc = _get_program()
    in_maps = [_make_in_map(rgb_map, ir_map, rgb_mask, ir_mask, n)
               for n in range(N)]
    res = run_bass_kernel_spmd(nc, in_maps, core_ids=list(range(NCORES)),
                               trace=trace, **trace_kw)
    return res.results, res


def finalize(results, rgb_mask, ir_mask):
    """Host-side -log / masked mean over the per-core row/col sums."""
    total = 0.0
    count = 0.0
    for n in range(N):
        rm = np.asarray(rgb_mask[n]).reshape(HW)
        im = np.asarray(ir_mask[n]).reshape(HW)
        rows = results[n]["rows"].astype(np.float64)
        cols = results[n]["cols"].astype(np.float64)
        row_lm = rows[:, :NK].T.reshape(HW)
        row_lg = rows[:, NK:].T.reshape(HW)
        col_lm = cols[0]
        col_lg = cols[1]
        row = row_lm / (row_lg + EPS)
        col = col_lm / (col_lg + EPS)
        for vec, mask in ((row, rm), (col, im)):
            v = vec * (mask > 0)
            nz = v != 0
            total += -np.log(v[nz]).sum()
            count += nz.sum()
    return np.float32(total / count)


def kernel(rgb_map, ir_map, rgb_mask, ir_mask):
    rgb_map = np.asarray(rgb_map, dtype=np.float32)
    ir_map = np.asarray(ir_map, dtype=np.float32)
    rgb_mask = np.asarray(rgb_mask, dtype=np.int32)
    ir_mask = np.asarray(ir_mask, dtype=np.int32)
    results, _ = run_device(rgb_map, ir_map, rgb_mask, ir_mask)
    return finalize(results, rgb_mask, ir_mask)


# revision 7
# speedup vs baseline: 1.1344x; 1.1344x over previous
"""Trainium2 Bass kernel for nn_CrossPixContrastiveL2.

Per sample (one per NeuronCore, N=8 samples / 8 cores):
  dist[p,q] = ||r_p||^2 + ||i_q||^2 - 2 r_p.i_q          (HW x HW, C=128)
  logit = exp(exp(-dist)/10)
  row[p] = sum_q logit*mask / (sum_q logit + eps)
  col[q] = sum_p logit*mask / (sum_p logit + eps)
  loss = masked mean of -log over foreground/nonzero entries

Device strategy per core:
  - bf16 Gram matmuls (K=C=128) into PSUM. A K=2 "broadcast" matmul first
    seeds PSUM with -||i_q||^2/2 (hi/lo bf16 split for f32-level accuracy),
    so PSUM = r.i - ||i||^2/2. The -||r_p||^2 term enters as the per-
    partition f32 bias of the first ACT pass.
  - ACT pass 1: e1 = Exp(2*PSUM - ||r||^2)   -> exp(-dist), bf16
  - ACT pass 2: logit = Exp(e1/10), fused accum_out -> row sums of logit
  - DVE scalar_tensor_tensor: (im_bcast == rm[p]) * logit with fused
    accum_out -> masked row sums (single op: mask+mul+reduce)
  - Column sums via label-onehot matmuls: lhsT = [onehot(rm) | ones]
    (128 x 22) against logit, accumulated over row tiles -> per-label
    column masses T[l,q]; col_lm[q] = T[im[q],q] via elementwise onehot
    select + K=22 matmul.
Host does the tiny final -log / masked mean over 4x1024 values per core.
"""

import os
from contextlib import ExitStack

import numpy as np
import ml_dtypes

import concourse.bacc as bacc
import concourse.tile as tile
import concourse.mybir as mybir
from concourse.bass_utils import run_bass_kernel_spmd

N, C, H, W = 8, 128, 32, 32
HW = H * W
NCORES = 8
NK = HW // 128          # 8 row tiles of 128 pixels
L = 21                  # label values 0..20
LL = L + 1              # onehot columns + ones column
TEMPERATURE = 10.0
EPS = 1e-6

_BF16 = ml_dtypes.bfloat16

_PROGRAM = None


def _build_program():
    f32 = mybir.dt.float32
    bf16 = mybir.dt.bfloat16
    AF = mybir.ActivationFunctionType
    ALU = mybir.AluOpType

    nc = bacc.Bacc("TRN2", target_bir_lowering=False, debug=False,
                   num_devices=NCORES)

    rgb = nc.dram_tensor("rgb", (C, HW), bf16, kind="ExternalInput").ap()
    irr = nc.dram_tensor("irr", (C, HW), bf16, kind="ExternalInput").ap()
    # hi/lo bf16 split of -||i_q||^2/2 (row0=hi, row1=lo)
    nihb = nc.dram_tensor("nihb", (2, HW), bf16, kind="ExternalInput").ap()
    # two rows of ones (lhsT for the K=2 broadcast matmul)
    ones2 = nc.dram_tensor("ones2", (2, 128), bf16, kind="ExternalInput").ap()
    # -||r_p||^2 in transposed layout [p, k] (ACT bias, f32 exact)
    nrT = nc.dram_tensor("nrT", (128, NK), f32, kind="ExternalInput").ap()
    # ir labels broadcast across partitions (bf16, exact for ints 0..20)
    imb = nc.dram_tensor("imb", (128, HW), bf16, kind="ExternalInput").ap()
    # [p, LL*k + l] = (rm[128k+p] == l) for l<21 ; 1.0 at l=21
    oh = nc.dram_tensor("oh", (128, NK * LL), bf16, kind="ExternalInput").ap()
    # rows 0..20 = onehot of im along q; row 21 = ones
    ohim = nc.dram_tensor("ohim", (LL, HW), f32, kind="ExternalInput").ap()
    # rm labels, transposed layout: [p, k] = rm[128k+p]
    rmf = nc.dram_tensor("rmf", (128, NK), f32, kind="ExternalInput").ap()
    # selector (22,2): col0 = 21 ones then 0 ; col1 = zeros then 1
    sel = nc.dram_tensor("sel", (LL, 2), f32, kind="ExternalInput").ap()

    # outputs: rows[:, 0:NK] = masked row sums, rows[:, NK:2NK] = row sums
    rows = nc.dram_tensor("rows", (128, 2 * NK), f32,
                          kind="ExternalOutput").ap()
    # cols[0] = masked col sums, cols[1] = col sums
    cols = nc.dram_tensor("cols", (2, HW), f32, kind="ExternalOutput").ap()

    with tile.TileContext(nc) as tc, ExitStack() as ctx:
        sb = ctx.enter_context(tc.tile_pool(name="sb", bufs=1))
        work = ctx.enter_context(tc.tile_pool(name="work", bufs=3))
        ps = ctx.enter_context(tc.tile_pool(name="ps", bufs=3, space="PSUM"))
        acc = ctx.enter_context(tc.tile_pool(name="acc", bufs=1, space="PSUM"))

        # ---- inputs; spread the DMA issues across idle engine queues and
        # order them by when the compute first needs each tensor.
        ir_s = sb.tile([C, HW], bf16)
        nc.sync.dma_start(ir_s[:, 0:512], irr[:, 0:512])
        nc.sync.dma_start(ir_s[:, 512:], irr[:, 512:])
        rgb_s = sb.tile([C, HW], bf16)
        nc.gpsimd.dma_start(rgb_s[:, 0:256], rgb[:, 0:256])
        nc.gpsimd.dma_start(rgb_s[:, 256:], rgb[:, 256:])
        nihb_s = sb.tile([2, HW], bf16)
        nc.sync.dma_start(nihb_s[:], nihb)
        ones2_s = sb.tile([2, 128], bf16)
        nc.sync.dma_start(ones2_s[:], ones2)
        nrT_s = sb.tile([128, NK], f32)
        nc.sync.dma_start(nrT_s[:], nrT)
        imb_s = sb.tile([128, HW], bf16)
        nc.scalar.dma_start(imb_s[:], imb)
        rmf_s = sb.tile([128, NK], f32)
        nc.gpsimd.dma_start(rmf_s[:], rmf)
        oh_s = sb.tile([128, NK * LL], bf16)
        nc.scalar.dma_start(oh_s[:], oh)
        ohim_s = sb.tile([LL, HW], f32)
        nc.scalar.dma_start(ohim_s[:], ohim)
        sel_s = sb.tile([LL, 2], f32)
        nc.scalar.dma_start(sel_s[:], sel)

        rows_s = sb.tile([128, 2 * NK], f32)
        # per-label column masses, accumulated across the NK row tiles
        TT = acc.tile([LL, HW], f32, tag="TT")

        for k in range(NK):
            G = ps.tile([128, HW], f32)
            for qh in range(2):
                q = qh * 512
                # PSUM <- -||i||^2/2 broadcast (K=2 hi/lo), then += r.i
                nc.tensor.matmul(G[:, q:q + 512],
                                 ones2_s[:],
                                 nihb_s[:, q:q + 512],
                                 start=True, stop=False)
                nc.tensor.matmul(G[:, q:q + 512],
                                 rgb_s[:, k * 128:(k + 1) * 128],
                                 ir_s[:, q:q + 512],
                                 start=False, stop=True)
            e1 = work.tile([128, HW], bf16, tag="e1")
            nc.scalar.activation(e1[:], G[:], AF.Exp, scale=2.0,
                                 bias=nrT_s[:, k:k + 1])
            logit = work.tile([128, HW], bf16, tag="logit")
            nc.scalar.activation(logit[:], e1[:], AF.Exp,
                                 scale=1.0 / TEMPERATURE,
                                 accum_out=rows_s[:, NK + k:NK + k + 1])
            lm = work.tile([128, HW], bf16, tag="lm")
            nc.vector.scalar_tensor_tensor(
                lm[:], imb_s[:], rmf_s[:, k:k + 1], logit[:],
                op0=ALU.is_equal, op1=ALU.mult,
                accum_out=rows_s[:, k:k + 1])
            for qh in range(2):
                q = qh * 512
                nc.tensor.matmul(TT[:, q:q + 512],
                                 oh_s[:, LL * k:LL * (k + 1)],
                                 logit[:, q:q + 512],
                                 start=(k == 0), stop=(k == NK - 1))

        # col_lm[q] = TT[im[q], q]; col_logit[q] = TT[21, q]
        TM = sb.tile([LL, HW], f32)
        nc.vector.tensor_tensor(TM[:], TT[:], ohim_s[:], op=ALU.mult)
        cps = acc.tile([2, HW], f32, tag="TT")
        for qh in range(2):
            q = qh * 512
            nc.tensor.matmul(cps[:, q:q + 512], sel_s[:], TM[:, q:q + 512],
                             start=True, stop=True)
        cols_s = sb.tile([2, HW], f32)
        nc.vector.tensor_copy(cols_s[:], cps[:])

        nc.sync.dma_start(rows, rows_s[:])
        nc.sync.dma_start(cols, cols_s[:])

    nc.compile()
    return nc


def _get_program():
    global _PROGRAM
    if _PROGRAM is None:
        _PROGRAM = _build_program()
    return _PROGRAM


def _make_in_map(rgb_map, ir_map, rgb_mask, ir_mask, n):
    f32 = np.float32
    rgb32 = np.ascontiguousarray(rgb_map[n].reshape(C, HW), dtype=f32)
    irr32 = np.ascontiguousarray(ir_map[n].reshape(C, HW), dtype=f32)
    rm = rgb_mask[n].reshape(HW)
    im = ir_mask[n].reshape(HW)

    nr = (rgb32 * rgb32).sum(axis=0, dtype=f32)
    ni = (irr32 * irr32).sum(axis=0, dtype=f32)

    x = (-0.5 * ni).astype(f32)
    hi = x.astype(_BF16)
    lo = (x - hi.astype(f32)).astype(_BF16)
    nihb = np.stack([hi, lo])

    ones2 = np.ones((2, 128), dtype=_BF16)

    rmT = rm.reshape(NK, 128).T  # [p, k]
    nrT = np.ascontiguousarray(-nr.reshape(NK, 128).T, dtype=f32)

    imb = np.broadcast_to(im.astype(_BF16), (128, HW)).copy()

    oh = np.zeros((128, NK, LL), dtype=_BF16)
    oh[:, :, :L] = (rmT[:, :, None] == np.arange(L)[None, None, :])
    oh[:, :, L] = 1
    oh = oh.reshape(128, NK * LL)

    ohim = np.zeros((LL, HW), dtype=f32)
    ohim[:L] = (np.arange(L)[:, None] == im[None, :])
    ohim[L] = 1.0

    rmf = np.ascontiguousarray(rmT, dtype=f32)

    sel = np.zeros((LL, 2), dtype=f32)
    sel[:L, 0] = 1.0
    sel[L, 1] = 1.0

    return {"rgb": rgb32.astype(_BF16), "irr": irr32.astype(_BF16),
            "nihb": nihb, "ones2": ones2, "nrT": nrT, "imb": imb,
            "oh": oh, "ohim": ohim, "rmf": rmf, "sel": sel}


def run_device(rgb_map, ir_map, rgb_mask, ir_mask, trace=False, **trace_kw):
    """Compile+run the SPMD kernel; returns (per-core results, BassKernelResults)."""
    nc = _get_program()
    in_maps = [_make_in_map(rgb_map, ir_map, rgb_mask, ir_mask, n)
               for n in range(N)]
    res = run_bass_kernel_spmd(nc, in_maps, core_ids=list(range(NCORES)),
                               trace=trace, **trace_kw)
    return res.results, res


def finalize(results, rgb_mask, ir_mask):
    """Host-side -log / masked mean over the per-core row/col sums."""
    total = 0.0
    count = 0.0
    for n in range(N):
        rm = np.asarray(rgb_mask[n]).reshape(HW)
        im = np.asarray(ir_mask[n]).reshape(HW)
        rows = results[n]["rows"].astype(np.float64)
        cols = results[n]["cols"].astype(np.float64)
        row_lm = rows[:, :NK].T.reshape(HW)
        row_lg = rows[:, NK:].T.reshape(HW)
        col_lm = cols[0]
        col_lg = cols[1]
        row = row_lm / (row_lg + EPS)
        col = col_lm / (col_lg + EPS)
        for vec, mask in ((row, rm), (col, im)):
            v = vec * (mask > 0)
            nz = v != 0
            total += -np.log(v[nz]).sum()
            count += nz.sum()
    return np.float32(total / count)


def kernel(rgb_map, ir_map, rgb_mask, ir_mask):
    rgb_map = np.asarray(rgb_map, dtype=np.float32)
    ir_map = np.asarray(ir_map, dtype=np.float32)
    rgb_mask = np.asarray(rgb_mask, dtype=np.int32)
    ir_mask = np.asarray(ir_mask, dtype=np.int32)
    results, _ = run_device(rgb_map, ir_map, rgb_mask, ir_mask)
    return finalize(results, rgb_mask, ir_mask)


# revision 8
# speedup vs baseline: 1.2963x; 1.1427x over previous
"""Trainium2 Bass kernel for nn_CrossPixContrastiveL2.

Per sample (one per NeuronCore, N=8 samples / 8 cores):
  dist[p,q] = ||r_p||^2 + ||i_q||^2 - 2 r_p.i_q          (HW x HW, C=128)
  logit = exp(exp(-dist)/10)
  row[p] = sum_q logit*mask / (sum_q logit + eps)
  col[q] = sum_p logit*mask / (sum_p logit + eps)
  loss = masked mean of -log over foreground/nonzero entries

Device strategy per core:
  - bf16 Gram matmuls (K=C=128) into PSUM. A K=2 "broadcast" matmul first
    seeds PSUM with -||i_q||^2/2 (hi/lo bf16 split for f32-level accuracy),
    so PSUM = r.i - ||i||^2/2. The -||r_p||^2 term enters as the per-
    partition f32 bias of the first ACT pass.
  - ACT pass 1: e1 = Exp(2*PSUM - ||r||^2)   -> exp(-dist), bf16
  - ACT pass 2: logit = Exp(e1/10), fused accum_out -> row sums of logit
  - DVE scalar_tensor_tensor: (im_bcast == rm[p]) * logit with fused
    accum_out -> masked row sums (single op: mask+mul+reduce)
  - Column sums via label-onehot matmuls: lhsT = [onehot(rm) | ones]
    (128 x 22) against logit, accumulated over row tiles -> per-label
    column masses T[l,q]; col_lm[q] = T[im[q],q] via elementwise onehot
    select + K=22 matmul.
Host does the tiny final -log / masked mean over 4x1024 values per core.
"""

import os
from contextlib import ExitStack

import numpy as np
import ml_dtypes

import concourse.bacc as bacc
import concourse.tile as tile
import concourse.mybir as mybir
from concourse.bass_utils import run_bass_kernel_spmd

N, C, H, W = 8, 128, 32, 32
HW = H * W
NCORES = 8
NK = HW // 128          # 8 row tiles of 128 pixels
L = 21                  # label values 0..20
LL = L + 1              # onehot columns + ones column
TEMPERATURE = 10.0
EPS = 1e-6

_BF16 = ml_dtypes.bfloat16

_PROGRAM = None


def _build_program():
    f32 = mybir.dt.float32
    bf16 = mybir.dt.bfloat16
    AF = mybir.ActivationFunctionType
    ALU = mybir.AluOpType

    nc = bacc.Bacc("TRN2", target_bir_lowering=False, debug=False,
                   num_devices=NCORES)

    rgb = nc.dram_tensor("rgb", (C, HW), bf16, kind="ExternalInput").ap()
    irr = nc.dram_tensor("irr", (C, HW), bf16, kind="ExternalInput").ap()
    # hi/lo bf16 split of -||i_q||^2/2 (row0=hi, row1=lo)
    nihb = nc.dram_tensor("nihb", (2, HW), bf16, kind="ExternalInput").ap()
    # two rows of ones (lhsT for the K=2 broadcast matmul)
    ones2 = nc.dram_tensor("ones2", (2, 128), bf16, kind="ExternalInput").ap()
    # -||r_p||^2 in transposed layout [p, k] (ACT bias, f32 exact)
    nrT = nc.dram_tensor("nrT", (128, NK), f32, kind="ExternalInput").ap()
    # ir labels as a single row (broadcast across partitions on device)
    imr = nc.dram_tensor("imr", (1, HW), bf16, kind="ExternalInput").ap()
    # [p, LL*k + l] = (rm[128k+p] == l) for l<21 ; 1.0 at l=21
    oh = nc.dram_tensor("oh", (128, NK * LL), bf16, kind="ExternalInput").ap()
    # rm labels, transposed layout: [p, k] = rm[128k+p]
    rmf = nc.dram_tensor("rmf", (128, NK), f32, kind="ExternalInput").ap()

    # outputs: rows[:, 0:NK] = masked row sums, rows[:, NK:2NK] = row sums
    rows = nc.dram_tensor("rows", (128, 2 * NK), f32,
                          kind="ExternalOutput").ap()
    # per-label column masses (host finishes the onehot select)
    ttd = nc.dram_tensor("ttd", (LL, HW), f32, kind="ExternalOutput").ap()

    with tile.TileContext(nc) as tc, ExitStack() as ctx:
        sb = ctx.enter_context(tc.tile_pool(name="sb", bufs=1))
        work = ctx.enter_context(tc.tile_pool(name="work", bufs=3))
        ps = ctx.enter_context(tc.tile_pool(name="ps", bufs=3, space="PSUM"))
        acc = ctx.enter_context(tc.tile_pool(name="acc", bufs=1, space="PSUM"))

        # ---- inputs; spread the DMA issues across idle engine queues and
        # order them by when the compute first needs each tensor.
        ir_s = sb.tile([C, HW], bf16)
        nc.sync.dma_start(ir_s[:, 0:512], irr[:, 0:512])
        nc.sync.dma_start(ir_s[:, 512:], irr[:, 512:])
        rgb_s = sb.tile([C, HW], bf16)
        nc.gpsimd.dma_start(rgb_s[:, 0:256], rgb[:, 0:256])
        nc.gpsimd.dma_start(rgb_s[:, 256:], rgb[:, 256:])
        nihb_s = sb.tile([2, HW], bf16)
        nc.scalar.dma_start(nihb_s[:], nihb)
        ones2_s = sb.tile([2, 128], bf16)
        nc.scalar.dma_start(ones2_s[:], ones2)
        nrT_s = sb.tile([128, NK], f32)
        nc.scalar.dma_start(nrT_s[:], nrT)
        imr_s = sb.tile([1, HW], bf16)
        nc.scalar.dma_start(imr_s[:], imr)
        rmf_s = sb.tile([128, NK], f32)
        nc.scalar.dma_start(rmf_s[:], rmf)
        oh_s = sb.tile([128, NK * LL], bf16)
        nc.scalar.dma_start(oh_s[:], oh)
        # broadcast the ir label row to all 128 partitions on the idle gpsimd
        imb_s = sb.tile([128, HW], bf16)
        nc.gpsimd.partition_broadcast(imb_s[:], imr_s[:], channels=128)

        rows_s = sb.tile([128, 2 * NK], f32)
        # per-label column masses, accumulated across the NK row tiles
        TT = acc.tile([LL, HW], f32, tag="TT")

        for k in range(NK):
            G = ps.tile([128, HW], f32)
            for qh in range(2):
                q = qh * 512
                # PSUM <- -||i||^2/2 broadcast (K=2 hi/lo), then += r.i
                nc.tensor.matmul(G[:, q:q + 512],
                                 ones2_s[:],
                                 nihb_s[:, q:q + 512],
                                 start=True, stop=False)
                nc.tensor.matmul(G[:, q:q + 512],
                                 rgb_s[:, k * 128:(k + 1) * 128],
                                 ir_s[:, q:q + 512],
                                 start=False, stop=True)
            e1 = work.tile([128, HW], bf16, tag="e1")
            nc.scalar.activation(e1[:], G[:], AF.Exp, scale=2.0,
                                 bias=nrT_s[:, k:k + 1])
            logit = work.tile([128, HW], bf16, tag="logit")
            nc.scalar.activation(logit[:], e1[:], AF.Exp,
                                 scale=1.0 / TEMPERATURE,
                                 accum_out=rows_s[:, NK + k:NK + k + 1])
            lm = work.tile([128, HW], bf16, tag="lm")
            nc.vector.scalar_tensor_tensor(
                lm[:], imb_s[:], rmf_s[:, k:k + 1], logit[:],
                op0=ALU.is_equal, op1=ALU.mult,
                accum_out=rows_s[:, k:k + 1])
            for qh in range(2):
                q = qh * 512
                nc.tensor.matmul(TT[:, q:q + 512],
                                 oh_s[:, LL * k:LL * (k + 1)],
                                 logit[:, q:q + 512],
                                 start=(k == 0), stop=(k == NK - 1))

        # ship the (22, HW) label masses to the host; it finishes the
        # per-column onehot select (col_lm[q] = TT[im[q],q], col_lg = TT[21])
        tts = sb.tile([LL, HW], f32)
        nc.vector.tensor_copy(tts[:], TT[:])

        nc.sync.dma_start(rows, rows_s[:])
        nc.sync.dma_start(ttd, tts[:])

    nc.compile()
    return nc


def _get_program():
    global _PROGRAM
    if _PROGRAM is None:
        _PROGRAM = _build_program()
    return _PROGRAM


def _make_in_map(rgb_map, ir_map, rgb_mask, ir_mask, n):
    f32 = np.float32
    rgb32 = np.ascontiguousarray(rgb_map[n].reshape(C, HW), dtype=f32)
    irr32 = np.ascontiguousarray(ir_map[n].reshape(C, HW), dtype=f32)
    rm = rgb_mask[n].reshape(HW)
    im = ir_mask[n].reshape(HW)

    nr = (rgb32 * rgb32).sum(axis=0, dtype=f32)
    ni = (irr32 * irr32).sum(axis=0, dtype=f32)

    x = (-0.5 * ni).astype(f32)
    hi = x.astype(_BF16)
    lo = (x - hi.astype(f32)).astype(_BF16)
    nihb = np.stack([hi, lo])

    ones2 = np.ones((2, 128), dtype=_BF16)

    rmT = rm.reshape(NK, 128).T  # [p, k]
    nrT = np.ascontiguousarray(-nr.reshape(NK, 128).T, dtype=f32)

    imr = im.astype(_BF16).reshape(1, HW)

    oh = np.zeros((128, NK, LL), dtype=_BF16)
    oh[:, :, :L] = (rmT[:, :, None] == np.arange(L)[None, None, :])
    oh[:, :, L] = 1
    oh = oh.reshape(128, NK * LL)

    rmf = np.ascontiguousarray(rmT, dtype=f32)

    return {"rgb": rgb32.astype(_BF16), "irr": irr32.astype(_BF16),
            "nihb": nihb, "ones2": ones2, "nrT": nrT, "imr": imr,
            "oh": oh, "rmf": rmf}


def run_device(rgb_map, ir_map, rgb_mask, ir_mask, trace=False, **trace_kw):
    """Compile+run the SPMD kernel; returns (per-core results, BassKernelResults)."""
    nc = _get_program()
    in_maps = [_make_in_map(rgb_map, ir_map, rgb_mask, ir_mask, n)
               for n in range(N)]
    res = run_bass_kernel_spmd(nc, in_maps, core_ids=list(range(NCORES)),
                               trace=trace, **trace_kw)
    return res.results, res


def finalize(results, rgb_mask, ir_mask):
    """Host-side -log / masked mean over the per-core row/col sums."""
    total = 0.0
    count = 0.0
    for n in range(N):
        rm = np.asarray(rgb_mask[n]).reshape(HW)
        im = np.asarray(ir_mask[n]).reshape(HW)
        rows = results[n]["rows"].astype(np.float64)
        tt = results[n]["ttd"].astype(np.float64)
        row_lm = rows[:, :NK].T.reshape(HW)
        row_lg = rows[:, NK:].T.reshape(HW)
        col_lm = tt[im, np.arange(HW)]
        col_lg = tt[L]
        row = row_lm / (row_lg + EPS)
        col = col_lm / (col_lg + EPS)
        for vec, mask in ((row, rm), (col, im)):
            v = vec * (mask > 0)
            nz = v != 0
            total += -np.log(v[nz]).sum()
            count += nz.sum()
    return np.float32(total / count)


def kernel(rgb_map, ir_map, rgb_mask, ir_mask):
    rgb_map = np.asarray(rgb_map, dtype=np.float32)
    ir_map = np.asarray(ir_map, dtype=np.float32)
    rgb_mask = np.asarray(rgb_mask, dtype=np.int32)
    ir_mask = np.asarray(ir_mask, dtype=np.int32)
    results, _ = run_device(rgb_map, ir_map, rgb_mask, ir_mask)
    return finalize(results, rgb_mask, ir_mask)


# revision 10
# speedup vs baseline: 1.4242x; 1.0987x over previous
"""Trainium2 Bass kernel for nn_CrossPixContrastiveL2.

Per sample (one per NeuronCore, N=8 samples / 8 cores):
  dist[p,q] = ||r_p||^2 + ||i_q||^2 - 2 r_p.i_q          (HW x HW, C=128)
  logit = exp(exp(-dist)/10)
  row[p] = sum_q logit*mask / (sum_q logit + eps)
  col[q] = sum_p logit*mask / (sum_p logit + eps)
  loss = masked mean of -log over foreground/nonzero entries

Device strategy per core:
  - bf16 Gram matmuls (K=C=128) into PSUM. A K=2 "broadcast" matmul first
    seeds PSUM with -||i_q||^2/2 (hi/lo bf16 split for f32-level accuracy),
    so PSUM = r.i - ||i||^2/2. The -||r_p||^2 term enters as the per-
    partition f32 bias of the first ACT pass.
  - ACT pass 1: e1 = Exp(2*PSUM - ||r||^2)   -> exp(-dist), bf16
  - ACT pass 2: logit = Exp(e1/10), fused accum_out -> row sums of logit
  - DVE scalar_tensor_tensor: (im_bcast == rm[p]) * logit with fused
    accum_out -> masked row sums (single op: mask+mul+reduce)
  - Column sums via label-onehot matmuls: lhsT = [onehot(rm) | ones]
    (128 x 22) against logit, accumulated over row tiles -> per-label
    column masses T[l,q]; col_lm[q] = T[im[q],q] via elementwise onehot
    select + K=22 matmul.
Host does the tiny final -log / masked mean over 4x1024 values per core.
"""

import os
from contextlib import ExitStack

import numpy as np
import ml_dtypes

import concourse.bacc as bacc
import concourse.tile as tile
import concourse.mybir as mybir
from concourse.bass_utils import run_bass_kernel_spmd

N, C, H, W = 8, 128, 32, 32
HW = H * W
NCORES = 8
NK = HW // 128          # 8 row tiles of 128 pixels
L = 21                  # label values 0..20
LL = L + 1              # onehot columns + ones column
TEMPERATURE = 10.0
EPS = 1e-6

_BF16 = ml_dtypes.bfloat16

_PROGRAM = None


def _build_program():
    f32 = mybir.dt.float32
    bf16 = mybir.dt.bfloat16
    AF = mybir.ActivationFunctionType
    ALU = mybir.AluOpType

    nc = bacc.Bacc("TRN2", target_bir_lowering=False, debug=False,
                   num_devices=NCORES)

    rgb = nc.dram_tensor("rgb", (C, HW), bf16, kind="ExternalInput").ap()
    irr = nc.dram_tensor("irr", (C, HW), bf16, kind="ExternalInput").ap()
    # hi/lo bf16 split of -||i_q||^2/2 (row0=hi, row1=lo)
    nihb = nc.dram_tensor("nihb", (2, HW), bf16, kind="ExternalInput").ap()
    # two rows of ones (lhsT for the K=2 broadcast matmul)
    ones2 = nc.dram_tensor("ones2", (2, 128), bf16, kind="ExternalInput").ap()
    # -||r_p||^2 in transposed layout [p, k] (ACT bias, f32 exact)
    nrT = nc.dram_tensor("nrT", (128, NK), f32, kind="ExternalInput").ap()
    # ir labels as a single row (broadcast across partitions on device)
    imr = nc.dram_tensor("imr", (1, HW), bf16, kind="ExternalInput").ap()
    # [p, LL*k + l] = (rm[128k+p] == l) for l<21 ; 1.0 at l=21
    oh = nc.dram_tensor("oh", (128, NK * LL), bf16, kind="ExternalInput").ap()
    # rm labels, transposed layout: [p, k] = rm[128k+p]
    rmf = nc.dram_tensor("rmf", (128, NK), f32, kind="ExternalInput").ap()

    # outputs: rows[:, 0:NK] = masked row sums, rows[:, NK:2NK] = row sums
    rows = nc.dram_tensor("rows", (128, 2 * NK), f32,
                          kind="ExternalOutput").ap()
    # per-label column masses, col-group packed: rows 0:22 = q<512,
    # rows 32:54 = q>=512 (host finishes the onehot select)
    ttd = nc.dram_tensor("ttd", (64, 512), f32, kind="ExternalOutput").ap()

    with tile.TileContext(nc) as tc, ExitStack() as ctx:
        sb = ctx.enter_context(tc.tile_pool(name="sb", bufs=1))
        work = ctx.enter_context(tc.tile_pool(name="work", bufs=3))
        ps = ctx.enter_context(tc.tile_pool(name="ps", bufs=3, space="PSUM"))
        acc = ctx.enter_context(tc.tile_pool(name="acc", bufs=1, space="PSUM"))

        # ---- inputs; spread the DMA issues across idle engine queues and
        # order them by when the compute first needs each tensor.
        ir_s = sb.tile([C, HW], bf16)
        nc.sync.dma_start(ir_s[:, 0:512], irr[:, 0:512])
        nc.sync.dma_start(ir_s[:, 512:], irr[:, 512:])
        rgb_s = sb.tile([C, HW], bf16)
        nc.gpsimd.dma_start(rgb_s[:, 0:256], rgb[:, 0:256])
        nc.gpsimd.dma_start(rgb_s[:, 256:], rgb[:, 256:])
        nihb_s = sb.tile([2, HW], bf16)
        nc.scalar.dma_start(nihb_s[:], nihb)
        ones2_s = sb.tile([2, 128], bf16)
        nc.scalar.dma_start(ones2_s[:], ones2)
        nrT_s = sb.tile([128, NK], f32)
        nc.scalar.dma_start(nrT_s[:], nrT)
        imr_s = sb.tile([1, HW], bf16)
        nc.scalar.dma_start(imr_s[:], imr)
        rmf_s = sb.tile([128, NK], f32)
        nc.scalar.dma_start(rmf_s[:], rmf)
        oh_s = sb.tile([128, NK * LL], bf16)
        nc.scalar.dma_start(oh_s[:], oh)
        # broadcast the ir label row to all 128 partitions on the idle gpsimd
        imb_s = sb.tile([128, HW], bf16)
        nc.gpsimd.partition_broadcast(imb_s[:], imr_s[:], channels=128)

        rows_s = sb.tile([128, 2 * NK], f32)
        # per-label column masses, accumulated across the NK row tiles.
        # Col-group packed into one PSUM bank: q-half 0 -> rows 0:22,
        # q-half 1 -> rows 32:54 (concurrent via PE column tiling).
        TT = acc.tile([64, 512], f32, tag="TT")

        for k in range(NK):
            G = ps.tile([128, HW], f32)
            for qh in range(2):
                q = qh * 512
                nc.tensor.matmul(G[:, q:q + 512],
                                 ones2_s[:],
                                 nihb_s[:, q:q + 512],
                                 start=True, stop=False)
            for qh in range(2):
                q = qh * 512
                nc.tensor.matmul(G[:, q:q + 512],
                                 rgb_s[:, k * 128:(k + 1) * 128],
                                 ir_s[:, q:q + 512],
                                 start=False, stop=True)
            e1 = work.tile([128, HW], bf16, tag="e1")
            nc.scalar.activation(e1[:], G[:], AF.Exp, scale=2.0,
                                 bias=nrT_s[:, k:k + 1])
            logit = work.tile([128, HW], bf16, tag="logit")
            nc.scalar.activation(logit[:], e1[:], AF.Exp,
                                 scale=1.0 / TEMPERATURE,
                                 accum_out=rows_s[:, NK + k:NK + k + 1])
            lm = work.tile([128, HW], bf16, tag="lm")
            nc.vector.scalar_tensor_tensor(
                lm[:], imb_s[:], rmf_s[:, k:k + 1], logit[:],
                op0=ALU.is_equal, op1=ALU.mult,
                accum_out=rows_s[:, k:k + 1])
            for qh in range(2):
                q = qh * 512
                nc.tensor.matmul(TT[32 * qh:32 * qh + LL, :],
                                 oh_s[:, LL * k:LL * (k + 1)],
                                 logit[:, q:q + 512],
                                 start=(k == 0), stop=(k == NK - 1),
                                 tile_position=(0, 32 * qh))

        # ship the packed label masses to the host; it finishes the
        # per-column onehot select (col_lm[q] = TT[im[q],q], col_lg = TT[21])
        tts = sb.tile([64, 512], f32)
        nc.vector.tensor_copy(tts[:], TT[:])

        nc.sync.dma_start(rows, rows_s[:])
        nc.sync.dma_start(ttd, tts[:])

    nc.compile()
    return nc


def _get_program():
    global _PROGRAM
    if _PROGRAM is None:
        _PROGRAM = _build_program()
    return _PROGRAM


def _make_in_map(rgb_map, ir_map, rgb_mask, ir_mask, n):
    f32 = np.float32
    rgb32 = np.ascontiguousarray(rgb_map[n].reshape(C, HW), dtype=f32)
    irr32 = np.ascontiguousarray(ir_map[n].reshape(C, HW), dtype=f32)
    rm = rgb_mask[n].reshape(HW)
    im = ir_mask[n].reshape(HW)

    nr = (rgb32 * rgb32).sum(axis=0, dtype=f32)
    ni = (irr32 * irr32).sum(axis=0, dtype=f32)

    x = (-0.5 * ni).astype(f32)
    hi = x.astype(_BF16)
    lo = (x - hi.astype(f32)).astype(_BF16)
    nihb = np.stack([hi, lo])

    ones2 = np.ones((2, 128), dtype=_BF16)

    rmT = rm.reshape(NK, 128).T  # [p, k]
    nrT = np.ascontiguousarray(-nr.reshape(NK, 128).T, dtype=f32)

    imr = im.astype(_BF16).reshape(1, HW)

    oh = np.zeros((128, NK, LL), dtype=_BF16)
    oh[:, :, :L] = (rmT[:, :, None] == np.arange(L)[None, None, :])
    oh[:, :, L] = 1
    oh = oh.reshape(128, NK * LL)

    rmf = np.ascontiguousarray(rmT, dtype=f32)

    return {"rgb": rgb32.astype(_BF16), "irr": irr32.astype(_BF16),
            "nihb": nihb, "ones2": ones2, "nrT": nrT, "imr": imr,
            "oh": oh, "rmf": rmf}


def run_device(rgb_map, ir_map, rgb_mask, ir_mask, trace=False, **trace_kw):
    """Compile+run the SPMD kernel; returns (per-core results, BassKernelResults)."""
    nc = _get_program()
    in_maps = [_make_in_map(rgb_map, ir_map, rgb_mask, ir_mask, n)
               for n in range(N)]
    res = run_bass_kernel_spmd(nc, in_maps, core_ids=list(range(NCORES)),
                               trace=trace, **trace_kw)
    return res.results, res


def finalize(results, rgb_mask, ir_mask):
    """Host-side -log / masked mean over the per-core row/col sums."""
    total = 0.0
    count = 0.0
    for n in range(N):
        rm = np.asarray(rgb_mask[n]).reshape(HW)
        im = np.asarray(ir_mask[n]).reshape(HW)
        rows = results[n]["rows"].astype(np.float64)
        ttp = results[n]["ttd"].astype(np.float64)
        tt = np.concatenate([ttp[0:LL, :], ttp[32:32 + LL, :]], axis=1)
        row_lm = rows[:, :NK].T.reshape(HW)
        row_lg = rows[:, NK:].T.reshape(HW)
        col_lm = tt[im, np.arange(HW)]
        col_lg = tt[L]
        row = row_lm / (row_lg + EPS)
        col = col_lm / (col_lg + EPS)
        for vec, mask in ((row, rm), (col, im)):
            v = vec * (mask > 0)
            nz = v != 0
            total += -np.log(v[nz]).sum()
            count += nz.sum()
    return np.float32(total / count)


def kernel(rgb_map, ir_map, rgb_mask, ir_mask):
    rgb_map = np.asarray(rgb_map, dtype=np.float32)
    ir_map = np.asarray(ir_map, dtype=np.float32)
    rgb_mask = np.asarray(rgb_mask, dtype=np.int32)
    ir_mask = np.asarray(ir_mask, dtype=np.int32)
    results, _ = run_device(rgb_map, ir_map, rgb_mask, ir_mask)
    return finalize(results, rgb_mask, ir_mask)


# revision 12
# speedup vs baseline: 1.4399x; 1.0110x over previous
"""Trainium2 Bass kernel for nn_CrossPixContrastiveL2.

Per sample (one per NeuronCore, N=8 samples / 8 cores):
  dist[p,q] = ||r_p||^2 + ||i_q||^2 - 2 r_p.i_q          (HW x HW, C=128)
  logit = exp(exp(-dist)/10)
  row[p] = sum_q logit*mask / (sum_q logit + eps)
  col[q] = sum_p logit*mask / (sum_p logit + eps)
  loss = masked mean of -log over foreground/nonzero entries

Device strategy per core:
  - bf16 Gram matmuls (K=C=128) into PSUM. A K=2 "broadcast" matmul first
    seeds PSUM with -||i_q||^2/2 (hi/lo bf16 split for f32-level accuracy),
    so PSUM = r.i - ||i||^2/2. The -||r_p||^2 term enters as the per-
    partition f32 bias of the first ACT pass.
  - ACT pass 1: e1 = Exp(2*PSUM - ||r||^2)   -> exp(-dist), bf16
  - ACT pass 2: logit = Exp(e1/10), fused accum_out -> row sums of logit
  - DVE scalar_tensor_tensor: (im_bcast == rm[p]) * logit with fused
    accum_out -> masked row sums (single op: mask+mul+reduce)
  - Column sums via label-onehot matmuls: lhsT = [onehot(rm) | ones]
    (128 x 22) against logit, accumulated over row tiles -> per-label
    column masses T[l,q]; col_lm[q] = T[im[q],q] via elementwise onehot
    select + K=22 matmul.
Host does the tiny final -log / masked mean over 4x1024 values per core.
"""

import os
from contextlib import ExitStack

import numpy as np
import ml_dtypes

import concourse.bacc as bacc
import concourse.tile as tile
import concourse.mybir as mybir
from concourse.bass_utils import run_bass_kernel_spmd

N, C, H, W = 8, 128, 32, 32
HW = H * W
NCORES = 8
NK = HW // 128          # 8 row tiles of 128 pixels
L = 21                  # label values 0..20
LL = L + 1              # onehot columns + ones column
TEMPERATURE = 10.0
EPS = 1e-6

_BF16 = ml_dtypes.bfloat16

_PROGRAM = None


def _build_program():
    f32 = mybir.dt.float32
    bf16 = mybir.dt.bfloat16
    AF = mybir.ActivationFunctionType
    ALU = mybir.AluOpType

    nc = bacc.Bacc("TRN2", target_bir_lowering=False, debug=False,
                   num_devices=NCORES)

    rgb = nc.dram_tensor("rgb", (C, HW), bf16, kind="ExternalInput").ap()
    irr = nc.dram_tensor("irr", (C, HW), bf16, kind="ExternalInput").ap()
    # hi/lo bf16 split of -||i_q||^2/2 (row0=hi, row1=lo)
    nihb = nc.dram_tensor("nihb", (2, HW), bf16, kind="ExternalInput").ap()
    # two rows of ones (lhsT for the K=2 broadcast matmul)
    ones2 = nc.dram_tensor("ones2", (2, 128), bf16, kind="ExternalInput").ap()
    # -||r_p||^2 in transposed layout [p, k] (ACT bias, f32 exact)
    nrT = nc.dram_tensor("nrT", (128, NK), f32, kind="ExternalInput").ap()
    # ir labels as a single row (broadcast across partitions on device)
    imr = nc.dram_tensor("imr", (1, HW), bf16, kind="ExternalInput").ap()
    # [p, LL*k + l] = (rm[128k+p] == l) for l<21 ; 1.0 at l=21
    oh = nc.dram_tensor("oh", (128, NK * LL), bf16, kind="ExternalInput").ap()
    # rm labels, transposed layout: [p, k] = rm[128k+p]
    rmf = nc.dram_tensor("rmf", (128, NK), f32, kind="ExternalInput").ap()

    # outputs: rows[:, 0:NK] = masked row sums, rows[:, NK:2NK] = row sums
    rows = nc.dram_tensor("rows", (128, 2 * NK), f32,
                          kind="ExternalOutput").ap()
    # per-label column masses, col-group packed: rows 0:22 = q<512,
    # rows 32:54 = q>=512 (host finishes the onehot select)
    ttd = nc.dram_tensor("ttd", (64, 512), f32, kind="ExternalOutput").ap()

    with tile.TileContext(nc) as tc, ExitStack() as ctx:
        sb = ctx.enter_context(tc.tile_pool(name="sb", bufs=1))
        work = ctx.enter_context(tc.tile_pool(name="work", bufs=3))
        ps = ctx.enter_context(tc.tile_pool(name="ps", bufs=3, space="PSUM"))
        acc = ctx.enter_context(tc.tile_pool(name="acc", bufs=1, space="PSUM"))

        # ---- inputs; spread the DMA issues across idle engine queues and
        # order them by when the compute first needs each tensor.
        ir_s = sb.tile([C, HW], bf16)
        nc.sync.dma_start(ir_s[:, 0:512], irr[:, 0:512])
        nc.sync.dma_start(ir_s[:, 512:], irr[:, 512:])
        rgb_s = sb.tile([C, HW], bf16)
        nc.gpsimd.dma_start(rgb_s[:, 0:256], rgb[:, 0:256])
        nc.gpsimd.dma_start(rgb_s[:, 256:], rgb[:, 256:])
        # hi/lo rows replicated at partitions 0:2 and 32:34 so the two
        # K=2 broadcast matmuls can use independent PE row strips
        nihb_s = sb.tile([34, HW], bf16)
        nc.scalar.dma_start(nihb_s[0:2, :], nihb)
        nc.scalar.dma_start(nihb_s[32:34, :], nihb)
        ones2_s = sb.tile([34, 128], bf16)
        nc.gpsimd.memset(ones2_s[:], 1.0)
        nrT_s = sb.tile([128, NK], f32)
        nc.scalar.dma_start(nrT_s[:], nrT)
        imr_s = sb.tile([1, HW], bf16)
        nc.scalar.dma_start(imr_s[:], imr)
        rmf_s = sb.tile([128, NK], f32)
        nc.scalar.dma_start(rmf_s[:], rmf)
        oh_s = sb.tile([128, NK * LL], bf16)
        nc.scalar.dma_start(oh_s[:], oh)
        # broadcast the ir label row to all 128 partitions on the idle gpsimd
        imb_s = sb.tile([128, HW], bf16)
        nc.gpsimd.partition_broadcast(imb_s[:], imr_s[:], channels=128)

        rows_s = sb.tile([128, 2 * NK], f32)
        # per-label column masses, accumulated across the NK row tiles.
        # Col-group packed into one PSUM bank: q-half 0 -> rows 0:22,
        # q-half 1 -> rows 32:54 (concurrent via PE column tiling).
        TT = acc.tile([64, 512], f32, tag="TT")

        for k in range(NK):
            G = ps.tile([128, HW], f32)
            for qh in range(2):
                q = qh * 512
                nc.tensor.matmul(G[:, q:q + 512],
                                 ones2_s[0:2, :],
                                 nihb_s[0:2, q:q + 512],
                                 start=True, stop=False)
            for qh in range(2):
                q = qh * 512
                nc.tensor.matmul(G[:, q:q + 512],
                                 rgb_s[:, k * 128:(k + 1) * 128],
                                 ir_s[:, q:q + 512],
                                 start=False, stop=True)
            e1 = work.tile([128, HW], bf16, tag="e1")
            nc.scalar.activation(e1[:], G[:], AF.Exp, scale=2.0,
                                 bias=nrT_s[:, k:k + 1])
            logit = work.tile([128, HW], bf16, tag="logit")
            nc.scalar.activation(logit[:], e1[:], AF.Exp,
                                 scale=1.0 / TEMPERATURE,
                                 accum_out=rows_s[:, NK + k:NK + k + 1])
            lm = work.tile([128, HW], bf16, tag="lm")
            nc.vector.scalar_tensor_tensor(
                lm[:], imb_s[:], rmf_s[:, k:k + 1], logit[:],
                op0=ALU.is_equal, op1=ALU.mult,
                accum_out=rows_s[:, k:k + 1])
            for qh in range(2):
                q = qh * 512
                nc.tensor.matmul(TT[32 * qh:32 * qh + LL, :],
                                 oh_s[:, LL * k:LL * (k + 1)],
                                 logit[:, q:q + 512],
                                 start=(k == 0), stop=(k == NK - 1),
                                 tile_position=(0, 32 * qh))

        # ship the packed label masses to the host; it finishes the
        # per-column onehot select (col_lm[q] = TT[im[q],q], col_lg = TT[21])
        tts = sb.tile([64, 512], f32)
        nc.vector.tensor_copy(tts[:], TT[:])

        nc.sync.dma_start(rows, rows_s[:])
        nc.sync.dma_start(ttd, tts[:])

    nc.compile()
    return nc


def _get_program():
    global _PROGRAM
    if _PROGRAM is None:
        _PROGRAM = _build_program()
    return _PROGRAM


def _make_in_map(rgb_map, ir_map, rgb_mask, ir_mask, n):
    f32 = np.float32
    rgb32 = np.ascontiguousarray(rgb_map[n].reshape(C, HW), dtype=f32)
    irr32 = np.ascontiguousarray(ir_map[n].reshape(C, HW), dtype=f32)
    rm = rgb_mask[n].reshape(HW)
    im = ir_mask[n].reshape(HW)

    nr = (rgb32 * rgb32).sum(axis=0, dtype=f32)
    ni = (irr32 * irr32).sum(axis=0, dtype=f32)

    x = (-0.5 * ni).astype(f32)
    hi = x.astype(_BF16)
    lo = (x - hi.astype(f32)).astype(_BF16)
    nihb = np.stack([hi, lo])

    ones2 = np.ones((2, 128), dtype=_BF16)

    rmT = rm.reshape(NK, 128).T  # [p, k]
    nrT = np.ascontiguousarray(-nr.reshape(NK, 128).T, dtype=f32)

    imr = im.astype(_BF16).reshape(1, HW)

    oh = np.zeros((128, NK, LL), dtype=_BF16)
    oh[:, :, :L] = (rmT[:, :, None] == np.arange(L)[None, None, :])
    oh[:, :, L] = 1
    oh = oh.reshape(128, NK * LL)

    rmf = np.ascontiguousarray(rmT, dtype=f32)

    return {"rgb": rgb32.astype(_BF16), "irr": irr32.astype(_BF16),
            "nihb": nihb, "ones2": ones2, "nrT": nrT, "imr": imr,
            "oh": oh, "rmf": rmf}


def run_device(rgb_map, ir_map, rgb_mask, ir_mask, trace=False, **trace_kw):
    """Compile+run the SPMD kernel; returns (per-core results, BassKernelResults)."""
    nc = _get_program()
    in_maps = [_make_in_map(rgb_map, ir_map, rgb_mask, ir_mask, n)
               for n in range(N)]
    res = run_bass_kernel_spmd(nc, in_maps, core_ids=list(range(NCORES)),
                               trace=trace, **trace_kw)
    return res.results, res


def finalize(results, rgb_mask, ir_mask):
    """Host-side -log / masked mean over the per-core row/col sums."""
    total = 0.0
    count = 0.0
    for n in range(N):
        rm = np.asarray(rgb_mask[n]).reshape(HW)
        im = np.asarray(ir_mask[n]).reshape(HW)
        rows = results[n]["rows"].astype(np.float64)
        ttp = results[n]["ttd"].astype(np.float64)
        tt = np.concatenate([ttp[0:LL, :], ttp[32:32 + LL, :]], axis=1)
        row_lm = rows[:, :NK].T.reshape(HW)
        row_lg = rows[:, NK:].T.reshape(HW)
        col_lm = tt[im, np.arange(HW)]
        col_lg = tt[L]
        row = row_lm / (row_lg + EPS)
        col = col_lm / (col_lg + EPS)
        for vec, mask in ((row, rm), (col, im)):
            v = vec * (mask > 0)
            nz = v != 0
            total += -np.log(v[nz]).sum()
            count += nz.sum()
    return np.float32(total / count)


def kernel(rgb_map, ir_map, rgb_mask, ir_mask):
    rgb_map = np.asarray(rgb_map, dtype=np.float32)
    ir_map = np.asarray(ir_map, dtype=np.float32)
    rgb_mask = np.asarray(rgb_mask, dtype=np.int32)
    ir_mask = np.asarray(ir_mask, dtype=np.int32)
    results, _ = run_device(rgb_map, ir_map, rgb_mask, ir_mask)
    return finalize(results, rgb_mask, ir_mask)
